# revision 8
# baseline (speedup 1.0000x reference)
"""DiffuserSelfAttention (sparse attention) Trainium2 Bass kernel.

Strategy: the edge-list graph attention is reformulated as dense masked
attention (density ~35%), head-parallel across the 8 NeuronCores (NH=8
heads, one head per core, zero collectives).

Per core (head h):
  1. qkT = [Wq_h/8 | Wk_h] @ hsT  (biases are zero per the spec)
  2. v [1024,64] (normal layout, i on partitions)
  3. St[j,i] = sum_d kT[d,j] qT[d,i]     (PE, K=64)
  4. Wt = exp(St) * adjmask              (ScalarE exp; mask mul split
     GpSimd/Vector so the exp pace is never blocked)
  5. 5 rounds: h <- 0.9 * (Wt^T h)/denom + 0.1 v ; denom comes from a
     ones column appended to h in round 0 (exact softmax denominator).
     Round 0 is interleaved with the score/exp pipeline per j-tile.

Schedule tuned from HW traces:
  - inputs go over TWO DMA rings (sync: wqkv+hsT, gpsimd: adjT as fp8)
    so the 1MB adjacency does not delay the projection-critical hsT;
  - the v projection runs AFTER the first two score tiles so the first
    exp (the phase pacer: 8x ~1.15us on Scalar) starts ~2us earlier;
  - Scalar does ONLY exp in that phase (qT/kT psum->sbuf copies are on
    Vector);
  - rounds 1..4 accumulate jt-major into two alternating half-round
    PSUM tiles, so the serialized finish chain of one round is fully
    covered by the next round's matmuls (no round-boundary PE stall);
  - each finish is a single fused scalar_tensor_tensor
    (dst = psum*recip + 0.1*v) on Vector.

All matmuls in bf16 (measured end-to-end rel err ~2.4e-3 vs f32 ref).

Self-contained: hardcodes B=1, S=1024, HIDDEN=512, NH=8, HD=64.
"""

import numpy as np
import ml_dtypes

S = 1024
HIDDEN = 512
NH = 8
HD = 64
P = 128
NT_S = S // P            # 8 node tiles
KDIM = HIDDEN            # contraction dim (biases are zero per the spec)
NT_K = KDIM // P         # 4 contraction tiles for projections
ALPHA = 0.1
N_ROUNDS = 5
WARMUP_MMS = 6          # dummy matmuls to warm the PE clock gate during DMA
R0_LAG = 4              # score->r0 software-pipeline depth (in half-tiles)

_CACHED = {}


def _build_module():
    import concourse.bass as bass
    import concourse.tile as tile
    from concourse import bacc
    import concourse.mybir as mybir

    f32 = mybir.dt.float32
    bf16 = mybir.dt.bfloat16
    fp8 = mybir.dt.float8e4
    AF = mybir.ActivationFunctionType
    ALU = mybir.AluOpType
    ts = bass.ts

    nc = bacc.Bacc("TRN2", target_bir_lowering=False, debug=False, num_devices=NH)

    hsT_d = nc.dram_tensor("hsT", [KDIM, S], bf16, kind="ExternalInput")
    wqkv_d = nc.dram_tensor("wqkv", [P, NT_K * (P + HD)], bf16, kind="ExternalInput")
    adjT_d = nc.dram_tensor("adjT", [S, S], fp8, kind="ExternalInput")
    out_d = nc.dram_tensor("out", [S, HD], f32, kind="ExternalOutput")

    hsT_t = hsT_d.ap().rearrange("(ko p) i -> p ko i", p=P)
    adjT_t = adjT_d.ap().rearrange("(t p) i -> p t i", p=P)
    out_t = out_d.ap().rearrange("(t p) d -> p t d", p=P)

    with tile.TileContext(nc) as tc:
        with (
            tc.tile_pool(name="singles", bufs=1) as singles,
            tc.tile_pool(name="psum_big", bufs=3, space="PSUM") as psum_big,
            tc.tile_pool(name="psum_small", bufs=2, space="PSUM") as psum_small,
        ):
            # ---- PE warmup: dummy matmuls on scratch while inputs DMA in ----
            scratch = singles.tile([P, 512], bf16)
            nc.vector.memset(scratch[:], 0.0)
            ps_w = psum_small.tile([P, 512], f32, name="ps_w", tag="ps_small")
            for _ in range(WARMUP_MMS):
                nc.tensor.matmul(
                    ps_w[:], scratch[:, :P], scratch[:], start=True, stop=True
                )

            # ---- load inputs on two DMA rings: sync gets the projection-
            # critical wqkv+hsT, gpsimd gets the (fp8) adjacency so the two
            # streams overlap instead of queueing. ----
            wqkv_sb = singles.tile([P, NT_K, P + HD], bf16)
            nc.sync.dma_start(
                wqkv_sb[:], wqkv_d.ap().rearrange("p (ko m) -> p ko m", ko=NT_K)
            )
            hsT_sb = singles.tile([P, NT_K, S], bf16)
            for ke in range(NT_K):
                nc.sync.dma_start(hsT_sb[:, ke, :], hsT_t[:, ke, :])
            adjT_sb = singles.tile([P, NT_S, S], fp8)
            for jc in range(4):
                nc.gpsimd.dma_start(
                    adjT_sb[:, 2 * jc : 2 * jc + 2, :], adjT_t[:, 2 * jc : 2 * jc + 2, :]
                )

            # ---- persistent intermediates ----
            qT_sb = singles.tile([HD, S], bf16)
            kT_sb = singles.tile([HD, S], bf16)
            wt_sb = singles.tile([P, NT_S, S], bf16)       # masked exp(score), [j, i]
            h0_sb = singles.tile([P, NT_S, HD + 1], bf16)  # v with ones column
            av_sb = singles.tile([P, NT_S, HD], f32)       # 0.1 * v
            h_a = singles.tile([P, NT_S, HD], bf16)
            h_b = singles.tile([P, NT_S, HD], bf16)
            recip_sb = singles.tile([P, NT_S, 1], f32)     # 0.9 / denom per i
            den_sb = singles.tile([P, NT_S, 1], f32)
            out_sb = singles.tile([P, NT_S, HD], f32)

            # ---- stage 1: q/k projections, ke-major so matmuls start as
            # each hsT k-tile lands from DMA.
            # Accumulation-group rule (verified on HW): start=True clears
            # group state BANK-wide, so within a shared bank only the very
            # first matmul carries start=True; bank-disjoint groups keep
            # normal start flags.
            nc.vector.memset(h0_sb[:, :, HD : HD + 1], 1.0)
            ps_q = psum_big.tile([HD, S], f32, name="ps_q", tag="ps_big")
            ps_k = psum_big.tile([HD, S], f32, name="ps_k", tag="ps_big")
            for ke in range(NT_K):
                for w0, ps in ((0, ps_q), (HD, ps_k)):
                    for n in range(2):
                        nc.tensor.matmul(
                            ps[:, ts(n, 512)],
                            wqkv_sb[:, ke, w0 : w0 + HD],
                            hsT_sb[:, ke, ts(n, 512)],
                            start=(ke == 0),
                            stop=(ke == NT_K - 1),
                            skip_group_check=True,
                        )
            # psum->sbuf copies off the Scalar engine (GpSimd cannot access
            # PSUM, so both go on Vector; Scalar stays free for exp).
            nc.vector.tensor_copy(out=kT_sb[:], in_=ps_k[:])
            nc.vector.tensor_copy(out=qT_sb[:], in_=ps_q[:])

            # ---- stage 3 + round 0, with the v projection slotted after
            # the first two score tiles: per j-tile score -> exp -> mask ->
            # (lagged) round-0 accumulate. ----
            ps_p0 = []
            for ih in range(2):
                ps = psum_small.tile(
                    [P, 4, HD + 1], f32, name=f"ps_p0_{ih}", tag="ps_small"
                )
                ps_p0.append(ps)

            def emit_r0(jt, n):
                # Groups interleave within each PSUM bank, so only the bank's
                # FIRST matmul carries start=True (start clears accumulation
                # state bank-wide — verified on HW).
                for il in range(4):
                    it = n * 4 + il
                    nc.tensor.matmul(
                        ps_p0[n][:, il, :],
                        wt_sb[:, jt, ts(it, P)],
                        h0_sb[:, jt, :],
                        start=(jt == 0 and il == 0),
                        stop=(jt == NT_S - 1 and il == 3),
                        skip_group_check=True,
                    )

            def emit_v():
                ps_vs = [
                    psum_small.tile([P, 4, HD], f32, name=f"ps_v{ih}", tag="ps_small")
                    for ih in range(2)
                ]
                for ke in range(NT_K):
                    for it in range(NT_S):
                        nc.tensor.matmul(
                            ps_vs[it // 4][:, it % 4, :],
                            hsT_sb[:, ke, ts(it, P)],
                            wqkv_sb[:, ke, P : P + HD],
                            start=(ke == 0 and it % 4 == 0),
                            stop=(ke == NT_K - 1 and it % 4 == 3),
                            skip_group_check=True,
                        )
                for ih in range(2):
                    sl = slice(ih * 4, ih * 4 + 4)
                    nc.vector.tensor_copy(out=h0_sb[:, sl, :HD], in_=ps_vs[ih][:])
                    nc.vector.tensor_scalar_mul(av_sb[:, sl, :], ps_vs[ih][:], ALPHA)

            lagged = []
            for jt in range(NT_S):
                ps_s = psum_big.tile([P, S], f32, name=f"ps_s{jt}", tag="ps_big")
                for n in range(2):
                    nc.tensor.matmul(
                        ps_s[:, ts(n, 512)],
                        kT_sb[:, ts(jt, P)],
                        qT_sb[:, ts(n, 512)],
                        start=True,
                        stop=True,
                    )
                # the first two j-tiles' masks go to GpSimd (slow but idle);
                # the rest to Vector so the tail tiles are never mask-gated
                mask_eng = nc.gpsimd if jt < 2 else nc.vector
                if jt < NT_S - 2:
                    nc.scalar.activation(
                        out=wt_sb[:, jt, :], in_=ps_s[:], func=AF.Exp
                    )
                    mask_eng.tensor_mul(
                        out=wt_sb[:, jt, :],
                        in0=wt_sb[:, jt, :],
                        in1=adjT_sb[:, jt, :],
                    )
                    for n in range(2):
                        lagged.append((jt, n))
                        if len(lagged) > R0_LAG:
                            emit_r0(*lagged.pop(0))
                else:
                    # last two j-tiles at half-width so their round-0 matmuls
                    # start per-half, shortening the drain before round 1
                    for n in range(2):
                        nc.scalar.activation(
                            out=wt_sb[:, jt, ts(n, 512)],
                            in_=ps_s[:, ts(n, 512)],
                            func=AF.Exp,
                        )
                        mask_eng.tensor_mul(
                            out=wt_sb[:, jt, ts(n, 512)],
                            in0=wt_sb[:, jt, ts(n, 512)],
                            in1=adjT_sb[:, jt, ts(n, 512)],
                        )
                        lagged.append((jt, n))
                        if len(lagged) > R0_LAG:
                            emit_r0(*lagged.pop(0))
                if jt == 1:
                    emit_v()
            while lagged:
                emit_r0(*lagged.pop(0))

            def finish_den(ps, sl):
                nc.vector.tensor_scalar_mul(
                    den_sb[:, sl, :], ps[:, :, HD : HD + 1], 1.0 / (1.0 - ALPHA)
                )
                nc.vector.reciprocal(recip_sb[:, sl, :], den_sb[:, sl, :])

            def finish_round(ps, base, sl, dst):
                """normalize + residual, one fused op per i-tile:
                  dst = (psum * recip) + av
                (recip is a per-partition [P,1] scalar AP)."""
                for il in range(sl.stop - sl.start):
                    it = sl.start + il
                    nc.vector.scalar_tensor_tensor(
                        out=dst[:, it, :],
                        in0=ps[:, base + il, :HD],
                        scalar=recip_sb[:, it, :],
                        in1=av_sb[:, it, :],
                        op0=ALU.mult,
                        op1=ALU.add,
                    )

            # hoist both halves' den/recip ahead of the 8 serialized finish
            # ops so round 1's jt-major consumption never waits on them
            for ih in range(2):
                finish_den(ps_p0[ih], slice(ih * 4, ih * 4 + 4))
            for ih in range(2):
                finish_round(ps_p0[ih], 0, slice(ih * 4, ih * 4 + 4), h_a)

            # ---- rounds 1..4: jt-major accumulation into two alternating
            # half-round PSUM tiles. jt-major means the next round's matmuls
            # consume the previous round's finish chain strictly slower than
            # it completes, so the PE never stalls at a round boundary.
            # (Tile tracks PSUM conflicts at tile granularity, hence two
            # tiles per round, not one.) ----
            for r in range(1, N_ROUNDS):
                h_cur = h_a if r % 2 == 1 else h_b
                h_next = h_b if r % 2 == 1 else h_a
                last = r == N_ROUNDS - 1
                dst = out_sb if last else h_next
                for ih in range(2):
                    ps_r = psum_small.tile(
                        [P, 4, HD], f32, name=f"ps_r{r}_{ih}", tag="ps_small"
                    )
                    for jt in range(NT_S):
                        for il in range(4):
                            it = ih * 4 + il
                            nc.tensor.matmul(
                                ps_r[:, il, :],
                                wt_sb[:, jt, ts(it, P)],
                                h_cur[:, jt, :],
                                start=(jt == 0 and il == 0),
                                stop=(jt == NT_S - 1),
                                skip_group_check=True,
                            )
                    sl = slice(ih * 4, ih * 4 + 4)
                    finish_round(ps_r, 0, sl, dst)
                    if last:
                        nc.sync.dma_start(out_t[:, sl, :], out_sb[:, sl, :])

    nc.compile()
    return nc


def _prep_inputs(hidden_states, attention_mask, Wq, bq, Wk, bk, Wv, bv, src, dst):
    bf = ml_dtypes.bfloat16
    f8 = ml_dtypes.float8_e4m3
    hs = np.asarray(hidden_states, np.float32).reshape(S, HIDDEN)
    scale = 1.0 / np.sqrt(HD)

    hsT = hs.T.astype(bf)

    WqT = np.asarray(Wq, np.float32).T * scale  # [HIDDEN, HIDDEN]
    WkT = np.asarray(Wk, np.float32).T
    WvT = np.asarray(Wv, np.float32).T
    # Zero biases per setup_inputs; the kernel folds no bias path, so be loud
    # if that assumption is ever violated rather than silently wrong.
    assert not (np.any(np.asarray(bq)) or np.any(np.asarray(bk)) or np.any(np.asarray(bv))), \
        "nonzero qkv biases are not supported by this kernel"

    # dense adjacency in [src, dst] layout, combined with the attention mask
    # (fp8 e4m3: 0.0 and 1.0 are exact)
    ok = (np.asarray(attention_mask, np.float32).reshape(S) > 0)
    adjT = np.zeros((S, S), np.float32)
    adjT[np.asarray(src), np.asarray(dst)] = 1.0
    adjT *= ok[:, None]
    adjT *= ok[None, :]
    adjT = adjT.astype(f8)

    in_maps = []
    for h in range(NH):
        sl = slice(h * HD, (h + 1) * HD)
        wqkv = np.zeros((KDIM, P + HD), np.float32)
        wqkv[:, :HD] = WqT[:, sl]
        wqkv[:, HD:P] = WkT[:, sl]
        wqkv[:, P:] = WvT[:, sl]
        wqkv_packed = (
            wqkv.reshape(NT_K, P, P + HD)
            .transpose(1, 0, 2)
            .reshape(P, NT_K * (P + HD))
        )
        in_maps.append(
            {
                "hsT": hsT,
                "wqkv": wqkv_packed.astype(bf),
                "adjT": adjT,
            }
        )
    return in_maps


def kernel(**inputs):
    from concourse.bass_utils import run_bass_kernel_spmd

    if "nc" not in _CACHED:
        _CACHED["nc"] = _build_module()
    nc = _CACHED["nc"]

    in_maps = _prep_inputs(**inputs)
    import os

    trace = bool(int(os.environ.get("KERNEL_TRACE", "0")))
    res = run_bass_kernel_spmd(
        nc,
        in_maps,
        core_ids=list(range(NH)),
        trace=trace,
        trace_cores=list(range(NH)) if trace else None,
    )
    _CACHED["last_results"] = res

    out = np.concatenate([res.results[h]["out"] for h in range(NH)], axis=1)
    return out.reshape(1, S, NH * HD).astype(np.float32)


# revision 9
# speedup vs baseline: 1.2333x; 1.2333x over previous
"""DiffuserSelfAttention (sparse attention) Trainium2 Bass kernel.

Strategy: the edge-list graph attention is reformulated as dense masked
attention (density ~35%), head-parallel across the 8 NeuronCores (NH=8
heads, one head per core, zero collectives).

Per core (head h):
  1. qkT = [Wq_h/8 | Wk_h] @ hsT  (biases are zero per the spec)
  2. v [1024,64] (normal layout, i on partitions)
  3. St[j,i] = sum_d kT[d,j] qT[d,i]     (PE, K=64)
  4. Wt = exp(St) * adjmask              (ScalarE exp; mask mul split
     GpSimd/Vector so the exp pace is never blocked)
  5. 5 rounds: h <- 0.9 * (Wt^T h)/denom + 0.1 v ; denom comes from a
     ones column appended to h in round 0 (exact softmax denominator).
     Round 0 is interleaved with the score/exp pipeline per j-tile.

Schedule tuned from HW traces:
  - inputs go over TWO DMA rings (sync: hsT then adjT, gpsimd: wqkv)
    so the projection-critical hsT starts streaming immediately;
  - the v projection runs AFTER the first two score tiles so the first
    exp (the phase pacer: 8x ~1.15us on Scalar) starts ~2us earlier;
  - Scalar does ONLY exp in that phase (qT/kT psum->sbuf copies are on
    Vector);
  - rounds 1..4 accumulate jt-major into two alternating half-round
    PSUM tiles, so the serialized finish chain of one round is fully
    covered by the next round's matmuls (no round-boundary PE stall);
  - each finish is a single fused scalar_tensor_tensor
    (dst = psum*recip + 0.1*v) on Vector.

All matmuls in bf16 (measured end-to-end rel err ~2.4e-3 vs f32 ref).

Self-contained: hardcodes B=1, S=1024, HIDDEN=512, NH=8, HD=64.
"""

import numpy as np
import ml_dtypes

S = 1024
HIDDEN = 512
NH = 8
HD = 64
P = 128
NT_S = S // P            # 8 node tiles
KDIM = HIDDEN            # contraction dim (biases are zero per the spec)
NT_K = KDIM // P         # 4 contraction tiles for projections
ALPHA = 0.1
N_ROUNDS = 5
WARMUP_MMS = 6          # dummy matmuls to warm the PE clock gate during DMA
R0_LAG = 4              # score->r0 software-pipeline depth (in half-tiles)

_CACHED = {}


def _build_module():
    import concourse.bass as bass
    import concourse.tile as tile
    from concourse import bacc
    import concourse.mybir as mybir

    f32 = mybir.dt.float32
    bf16 = mybir.dt.bfloat16
    fp8 = mybir.dt.float8e4
    AF = mybir.ActivationFunctionType
    ALU = mybir.AluOpType
    ts = bass.ts

    nc = bacc.Bacc("TRN2", target_bir_lowering=False, debug=False, num_devices=NH)

    hsT_d = nc.dram_tensor("hsT", [KDIM, S], bf16, kind="ExternalInput")
    wqkv_d = nc.dram_tensor("wqkv", [P, NT_K * (P + HD)], bf16, kind="ExternalInput")
    adjT_d = nc.dram_tensor("adjT", [S, S], bf16, kind="ExternalInput")
    out_d = nc.dram_tensor("out", [S, HD], f32, kind="ExternalOutput")

    hsT_t = hsT_d.ap().rearrange("(ko p) i -> p ko i", p=P)
    adjT_t = adjT_d.ap().rearrange("(t p) i -> p t i", p=P)
    out_t = out_d.ap().rearrange("(t p) d -> p t d", p=P)

    with tile.TileContext(nc) as tc:
        with (
            tc.tile_pool(name="singles", bufs=1) as singles,
            tc.tile_pool(name="psum_big", bufs=3, space="PSUM") as psum_big,
            tc.tile_pool(name="psum_small", bufs=2, space="PSUM") as psum_small,
        ):
            # ---- PE warmup: dummy matmuls on scratch while inputs DMA in ----
            scratch = singles.tile([P, 512], bf16)
            nc.vector.memset(scratch[:], 0.0)
            ps_w = psum_small.tile([P, 512], f32, name="ps_w", tag="ps_small")
            for _ in range(WARMUP_MMS):
                nc.tensor.matmul(
                    ps_w[:], scratch[:, :P], scratch[:], start=True, stop=True
                )

            # ---- load inputs on two DMA rings: sync gets the projection-
            # critical wqkv+hsT, gpsimd gets the (fp8) adjacency so the two
            # streams overlap instead of queueing. ----
            wqkv_sb = singles.tile([P, NT_K, P + HD], bf16)
            nc.gpsimd.dma_start(
                wqkv_sb[:], wqkv_d.ap().rearrange("p (ko m) -> p ko m", ko=NT_K)
            )
            hsT_sb = singles.tile([P, NT_K, S], bf16)
            for ke in range(NT_K):
                nc.sync.dma_start(hsT_sb[:, ke, :], hsT_t[:, ke, :])
            adjT_sb = singles.tile([P, NT_S, S], bf16)
            for jc in range(4):
                nc.sync.dma_start(
                    adjT_sb[:, 2 * jc : 2 * jc + 2, :], adjT_t[:, 2 * jc : 2 * jc + 2, :]
                )

            # ---- persistent intermediates ----
            qT_sb = singles.tile([HD, S], bf16)
            kT_sb = singles.tile([HD, S], bf16)
            wt_sb = singles.tile([P, NT_S, S], bf16)       # masked exp(score), [j, i]
            h0_sb = singles.tile([P, NT_S, HD + 1], bf16)  # v with ones column
            av_sb = singles.tile([P, NT_S, HD], f32)       # 0.1 * v
            h_a = singles.tile([P, NT_S, HD], bf16)
            h_b = singles.tile([P, NT_S, HD], bf16)
            recip_sb = singles.tile([P, NT_S, 1], f32)     # 0.9 / denom per i
            den_sb = singles.tile([P, NT_S, 1], f32)
            out_sb = singles.tile([P, NT_S, HD], f32)

            # ---- stage 1: q/k projections, ke-major so matmuls start as
            # each hsT k-tile lands from DMA.
            # Accumulation-group rule (verified on HW): start=True clears
            # group state BANK-wide, so within a shared bank only the very
            # first matmul carries start=True; bank-disjoint groups keep
            # normal start flags.
            nc.vector.memset(h0_sb[:, :, HD : HD + 1], 1.0)
            ps_q = psum_big.tile([HD, S], f32, name="ps_q", tag="ps_big")
            ps_k = psum_big.tile([HD, S], f32, name="ps_k", tag="ps_big")
            for ke in range(NT_K):
                for w0, ps in ((0, ps_q), (HD, ps_k)):
                    for n in range(2):
                        nc.tensor.matmul(
                            ps[:, ts(n, 512)],
                            wqkv_sb[:, ke, w0 : w0 + HD],
                            hsT_sb[:, ke, ts(n, 512)],
                            start=(ke == 0),
                            stop=(ke == NT_K - 1),
                            skip_group_check=True,
                        )
            # kT on Vector, qT on Scalar: the two psum->sbuf copies run in
            # parallel, and Scalar is otherwise idle until the first exp.
            nc.vector.tensor_copy(out=kT_sb[:], in_=ps_k[:])
            nc.scalar.copy(out=qT_sb[:], in_=ps_q[:])

            # ---- stage 3 + round 0, with the v projection slotted after
            # the first two score tiles: per j-tile score -> exp -> mask ->
            # (lagged) round-0 accumulate. ----
            ps_p0 = []
            for ih in range(2):
                ps = psum_small.tile(
                    [P, 4, HD + 1], f32, name=f"ps_p0_{ih}", tag="ps_small"
                )
                ps_p0.append(ps)

            def emit_r0(jt, n):
                # Groups interleave within each PSUM bank, so only the bank's
                # FIRST matmul carries start=True (start clears accumulation
                # state bank-wide — verified on HW).
                for il in range(4):
                    it = n * 4 + il
                    nc.tensor.matmul(
                        ps_p0[n][:, il, :],
                        wt_sb[:, jt, ts(it, P)],
                        h0_sb[:, jt, :],
                        start=(jt == 0 and il == 0),
                        stop=(jt == NT_S - 1 and il == 3),
                        skip_group_check=True,
                    )

            def emit_v():
                ps_vs = [
                    psum_small.tile([P, 4, HD], f32, name=f"ps_v{ih}", tag="ps_small")
                    for ih in range(2)
                ]
                for ke in range(NT_K):
                    for it in range(NT_S):
                        nc.tensor.matmul(
                            ps_vs[it // 4][:, it % 4, :],
                            hsT_sb[:, ke, ts(it, P)],
                            wqkv_sb[:, ke, P : P + HD],
                            start=(ke == 0 and it % 4 == 0),
                            stop=(ke == NT_K - 1 and it % 4 == 3),
                            skip_group_check=True,
                        )
                for ih in range(2):
                    sl = slice(ih * 4, ih * 4 + 4)
                    nc.vector.tensor_copy(out=h0_sb[:, sl, :HD], in_=ps_vs[ih][:])
                    nc.vector.tensor_scalar_mul(av_sb[:, sl, :], ps_vs[ih][:], ALPHA)

            lagged = []
            for jt in range(NT_S):
                ps_s = psum_big.tile([P, S], f32, name=f"ps_s{jt}", tag="ps_big")
                for n in range(2):
                    nc.tensor.matmul(
                        ps_s[:, ts(n, 512)],
                        kT_sb[:, ts(jt, P)],
                        qT_sb[:, ts(n, 512)],
                        start=True,
                        stop=True,
                    )
                # the first two j-tiles' masks go to GpSimd (slow but idle);
                # the rest to Vector so the tail tiles are never mask-gated
                mask_eng = nc.gpsimd if jt < 2 else nc.vector
                if jt < NT_S - 2:
                    nc.scalar.activation(
                        out=wt_sb[:, jt, :], in_=ps_s[:], func=AF.Exp
                    )
                    mask_eng.tensor_mul(
                        out=wt_sb[:, jt, :],
                        in0=wt_sb[:, jt, :],
                        in1=adjT_sb[:, jt, :],
                    )
                    for n in range(2):
                        lagged.append((jt, n))
                        if len(lagged) > R0_LAG:
                            emit_r0(*lagged.pop(0))
                else:
                    # last two j-tiles at half-width so their round-0 matmuls
                    # start per-half, shortening the drain before round 1
                    for n in range(2):
                        nc.scalar.activation(
                            out=wt_sb[:, jt, ts(n, 512)],
                            in_=ps_s[:, ts(n, 512)],
                            func=AF.Exp,
                        )
                        mask_eng.tensor_mul(
                            out=wt_sb[:, jt, ts(n, 512)],
                            in0=wt_sb[:, jt, ts(n, 512)],
                            in1=adjT_sb[:, jt, ts(n, 512)],
                        )
                        lagged.append((jt, n))
                        if len(lagged) > R0_LAG:
                            emit_r0(*lagged.pop(0))
                if jt == 1:
                    emit_v()
            while lagged:
                emit_r0(*lagged.pop(0))

            def finish_den(ps, sl):
                nc.vector.tensor_scalar_mul(
                    den_sb[:, sl, :], ps[:, :, HD : HD + 1], 1.0 / (1.0 - ALPHA)
                )
                nc.vector.reciprocal(recip_sb[:, sl, :], den_sb[:, sl, :])

            def finish_round(ps, base, sl, dst):
                """normalize + residual, one fused op per i-tile:
                  dst = (psum * recip) + av
                (recip is a per-partition [P,1] scalar AP)."""
                for il in range(sl.stop - sl.start):
                    it = sl.start + il
                    nc.vector.scalar_tensor_tensor(
                        out=dst[:, it, :],
                        in0=ps[:, base + il, :HD],
                        scalar=recip_sb[:, it, :],
                        in1=av_sb[:, it, :],
                        op0=ALU.mult,
                        op1=ALU.add,
                    )

            # hoist both halves' den/recip ahead of the 8 serialized finish
            # ops so round 1's jt-major consumption never waits on them
            for ih in range(2):
                finish_den(ps_p0[ih], slice(ih * 4, ih * 4 + 4))
            for ih in range(2):
                finish_round(ps_p0[ih], 0, slice(ih * 4, ih * 4 + 4), h_a)

            # ---- rounds 1..4: jt-major accumulation into two alternating
            # half-round PSUM tiles. jt-major means the next round's matmuls
            # consume the previous round's finish chain strictly slower than
            # it completes, so the PE never stalls at a round boundary.
            # (Tile tracks PSUM conflicts at tile granularity, hence two
            # tiles per round, not one.) ----
            for r in range(1, N_ROUNDS):
                h_cur = h_a if r % 2 == 1 else h_b
                h_next = h_b if r % 2 == 1 else h_a
                last = r == N_ROUNDS - 1
                dst = out_sb if last else h_next
                for ih in range(2):
                    ps_r = psum_small.tile(
                        [P, 4, HD], f32, name=f"ps_r{r}_{ih}", tag="ps_small"
                    )
                    for jt in range(NT_S):
                        for il in range(4):
                            it = ih * 4 + il
                            nc.tensor.matmul(
                                ps_r[:, il, :],
                                wt_sb[:, jt, ts(it, P)],
                                h_cur[:, jt, :],
                                start=(jt == 0 and il == 0),
                                stop=(jt == NT_S - 1),
                                skip_group_check=True,
                            )
                    sl = slice(ih * 4, ih * 4 + 4)
                    finish_round(ps_r, 0, sl, dst)
                    if last:
                        nc.sync.dma_start(out_t[:, sl, :], out_sb[:, sl, :])

    nc.compile()
    return nc


def _prep_inputs(hidden_states, attention_mask, Wq, bq, Wk, bk, Wv, bv, src, dst):
    bf = ml_dtypes.bfloat16
    f8 = ml_dtypes.float8_e4m3
    hs = np.asarray(hidden_states, np.float32).reshape(S, HIDDEN)
    scale = 1.0 / np.sqrt(HD)

    hsT = hs.T.astype(bf)

    WqT = np.asarray(Wq, np.float32).T * scale  # [HIDDEN, HIDDEN]
    WkT = np.asarray(Wk, np.float32).T
    WvT = np.asarray(Wv, np.float32).T
    # Zero biases per setup_inputs; the kernel folds no bias path, so be loud
    # if that assumption is ever violated rather than silently wrong.
    assert not (np.any(np.asarray(bq)) or np.any(np.asarray(bk)) or np.any(np.asarray(bv))), \
        "nonzero qkv biases are not supported by this kernel"

    # dense adjacency in [src, dst] layout, combined with the attention mask
    # (fp8 e4m3: 0.0 and 1.0 are exact)
    ok = (np.asarray(attention_mask, np.float32).reshape(S) > 0)
    adjT = np.zeros((S, S), np.float32)
    adjT[np.asarray(src), np.asarray(dst)] = 1.0
    adjT *= ok[:, None]
    adjT *= ok[None, :]
    adjT = adjT.astype(bf)

    in_maps = []
    for h in range(NH):
        sl = slice(h * HD, (h + 1) * HD)
        wqkv = np.zeros((KDIM, P + HD), np.float32)
        wqkv[:, :HD] = WqT[:, sl]
        wqkv[:, HD:P] = WkT[:, sl]
        wqkv[:, P:] = WvT[:, sl]
        wqkv_packed = (
            wqkv.reshape(NT_K, P, P + HD)
            .transpose(1, 0, 2)
            .reshape(P, NT_K * (P + HD))
        )
        in_maps.append(
            {
                "hsT": hsT,
                "wqkv": wqkv_packed.astype(bf),
                "adjT": adjT,
            }
        )
    return in_maps


def kernel(**inputs):
    from concourse.bass_utils import run_bass_kernel_spmd

    if "nc" not in _CACHED:
        _CACHED["nc"] = _build_module()
    nc = _CACHED["nc"]

    in_maps = _prep_inputs(**inputs)
    import os

    trace = bool(int(os.environ.get("KERNEL_TRACE", "0")))
    res = run_bass_kernel_spmd(
        nc,
        in_maps,
        core_ids=list(range(NH)),
        trace=trace,
        trace_cores=list(range(NH)) if trace else None,
    )
    _CACHED["last_results"] = res

    out = np.concatenate([res.results[h]["out"] for h in range(NH)], axis=1)
    return out.reshape(1, S, NH * HD).astype(np.float32)
